# revision 1
# baseline (speedup 1.0000x reference)
"""Cosine-similarity batch attention on 8 TRN2 NeuronCores.

reference:  xn = x / ||x||_row;  out = softmax(xn @ xn.T, axis=-1) @ x
x: [8192, 512] fp32.

Sharding: rows of x (the query dim) are split across the 8 cores; every core
holds the full x for the key/value side.  Per core (SPMD program):

  prep:  load x tiles [128, 512] fp32, row-norms via ACT(Square, accum_out)
         + Sqrt + DVE reciprocal, scale to fp16 (xn), XBAR dma-transpose into
         xnT chunks [128C, 8192k]; V = x cast to fp16 kept in natural layout.
         Same for this core's 1024 query rows -> qnT [128C, 1024q].
  main:  flash-attention style over 64 k-blocks per 512-wide q-block:
         ST[k,q]  = sum_c xnT[c,kblk].T @ qnT[c,qblk]    (PSUM fp32)
         E        = exp(ST)                              (ACT, fp16, scores
                                                          are cosines in
                                                          [-1,1] so no max
                                                          subtraction needed)
         O[q,C]  += E[:,qsub].T @ V[kblk]                (PSUM accum fp32)
         rs[q]   += E[:,qsub].T @ ones                   (softmax denominator)
  epi:   out = O * (1/rs), fp32, DMA to DRAM.

All matmul operands fp16 (PE full rate), all accumulation fp32.
"""

import numpy as np

B, C = 8192, 512
M = 8                 # cores
QB = B // M           # 1024 query rows per core
P = 128               # SBUF partitions
NK = B // P           # 64 k-blocks
QBLK = 512            # q-block width (one PSUM bank of fp32)
NQB = QB // QBLK      # 2 q-blocks per core
NSUB = QBLK // P      # 4 q sub-slices per q-block (matmul M<=128)
CCH = C // P          # 4 contraction chunks of 128

_cached_nc = None


def _build():
    import concourse.bacc as bacc
    import concourse.tile as tile
    from concourse import mybir

    f32 = mybir.dt.float32
    f16 = mybir.dt.float16
    Act = mybir.ActivationFunctionType

    nc = bacc.Bacc("TRN2", target_bir_lowering=False, debug=False, num_devices=M)
    x = nc.dram_tensor("x", [B, C], f32, kind="ExternalInput").ap()
    xq = nc.dram_tensor("xq", [QB, C], f32, kind="ExternalInput").ap()
    out = nc.dram_tensor("out", [QB, C], f32, kind="ExternalOutput").ap()

    with tile.TileContext(nc) as tc:
        with (
            tc.tile_pool(name="resident", bufs=1) as resident,
            tc.tile_pool(name="io", bufs=4) as io,
            tc.tile_pool(name="work", bufs=4) as work,
            tc.tile_pool(name="epi", bufs=3) as epi,
            tc.tile_pool(name="st_psum", bufs=2, space="PSUM") as st_psum,
            tc.tile_pool(name="o_psum", bufs=1, space="PSUM") as o_psum,
            tc.tile_pool(name="rs_psum", bufs=1, space="PSUM") as rs_psum,
        ):
            # resident fp16 operand buffers
            xnT = [resident.tile([P, B], f16, name=f"xnT{c}") for c in range(CCH)]
            qnT = [resident.tile([P, QB], f16, name=f"qnT{c}") for c in range(CCH)]
            v = resident.tile([P, NK, C], f16, name="v")
            ones = resident.tile([P, 1], f16, name="ones")
            nc.vector.memset(ones, 1.0)

            def prep_tile(src_rows, dest_chunks, col, v_dest=None):
                """Normalize 128 rows of src and transpose into dest chunks."""
                xt = io.tile([P, C], f32, tag="xload")
                nc.sync.dma_start(out=xt, in_=src_rows)
                sq = work.tile([P, C], f32, tag="sq")
                ssq = work.tile([P, 1], f32, tag="ssq")
                nc.scalar.activation(out=sq, in_=xt, func=Act.Square, accum_out=ssq)
                nrm = work.tile([P, 1], f32, tag="nrm")
                nc.scalar.activation(out=nrm, in_=ssq, func=Act.Sqrt)
                rnorm = work.tile([P, 1], f32, tag="rnorm")
                nc.vector.reciprocal(out=rnorm, in_=nrm)
                xnf = work.tile([P, C], f16, tag="xnf")
                nc.vector.tensor_scalar_mul(out=xnf, in0=xt, scalar1=rnorm)
                if v_dest is not None:
                    nc.gpsimd.tensor_copy(out=v_dest, in_=xt)
                for c in range(CCH):
                    nc.sync.dma_start_transpose(
                        out=dest_chunks[c][:, col : col + P],
                        in_=xnf[:, c * P : (c + 1) * P],
                    )

            # query side first so the main loop can start early
            for t in range(QB // P):
                prep_tile(xq[t * P : (t + 1) * P, :], qnT, t * P)
            for kb in range(NK):
                prep_tile(x[kb * P : (kb + 1) * P, :], xnT, kb * P, v_dest=v[:, kb, :])

            for qb in range(NQB):
                o_ps = o_psum.tile([P, NSUB, C], f32, tag="o")
                rs_ps = rs_psum.tile([P, NSUB], f32, tag="rs")
                for kb in range(NK):
                    st = st_psum.tile([P, QBLK], f32, tag="st")
                    for c in range(CCH):
                        nc.tensor.matmul(
                            st,
                            lhsT=xnT[c][:, kb * P : (kb + 1) * P],
                            rhs=qnT[c][:, qb * QBLK : (qb + 1) * QBLK],
                            start=(c == 0),
                            stop=(c == CCH - 1),
                        )
                    est = work.tile([P, QBLK], f16, tag="est")
                    nc.scalar.activation(out=est, in_=st, func=Act.Exp)
                    for s in range(NSUB):
                        nc.tensor.matmul(
                            o_ps[:, s, :],
                            lhsT=est[:, s * P : (s + 1) * P],
                            rhs=v[:, kb, :],
                            start=(kb == 0),
                            stop=(kb == NK - 1),
                        )
                        nc.tensor.matmul(
                            rs_ps[:, s : s + 1],
                            lhsT=est[:, s * P : (s + 1) * P],
                            rhs=ones,
                            start=(kb == 0),
                            stop=(kb == NK - 1),
                        )
                recip = epi.tile([P, NSUB], f32, tag="recip")
                nc.vector.reciprocal(out=recip, in_=rs_ps)
                for s in range(NSUB):
                    oo = epi.tile([P, C], f32, tag="oout")
                    nc.vector.tensor_scalar_mul(
                        out=oo, in0=o_ps[:, s, :], scalar1=recip[:, s : s + 1]
                    )
                    r0 = qb * QBLK + s * P
                    nc.sync.dma_start(out=out[r0 : r0 + P, :], in_=oo)

    nc.compile()
    return nc


def kernel(**inputs):
    global _cached_nc
    from concourse import bass_utils

    x = np.ascontiguousarray(np.asarray(inputs["x"], dtype=np.float32))
    if _cached_nc is None:
        _cached_nc = _build()
    in_maps = [{"x": x, "xq": x[i * QB : (i + 1) * QB]} for i in range(M)]
    res = bass_utils.run_bass_kernel_spmd(_cached_nc, in_maps, core_ids=list(range(M)))
    return np.concatenate([res.results[i]["out"] for i in range(M)], axis=0)
